# revision 4
# baseline (speedup 1.0000x reference)
"""LoRA SwiGLU MLP on 8 Trainium2 NeuronCores — DP-8, startup-optimized v2.

Data-parallel: LoRA folded on host, each core computes 512 tokens against
full folded bf16 weights, no collectives. PE cadence is 215.83 ns per
128x128x512 matmul (8256 matmuls/core = 1782 us floor); everything else
is edge trimming:
  - 12 wide warmup matmuls issued immediately (dep only on a tiny memset)
    flip the HAM clock gate (~5 us cold busy) while DMA queues kick in.
  - Pivoted first wave: h=0 and h=1 (gate+up) are computed k-major in 4
    interleaved PSUM groups, so each x k-slice (128 KB) feeds 4 matmuls
    (~64 KB of new bytes per matmul ~= 296 GB/s at full clock) and the PE
    never waits on the 4 MB x tensor landing.
  - Fine DMA granularity at the head: x in 16 chunks of [P,2,M], first
    weights in [P,4,P] sub-tiles interleaved by k-quad.
  - Tail: the last down-tile PSUM drain alternates vector/scalar engines.
"""

import numpy as np
import ml_dtypes

import concourse.mybir as mybir
import concourse.tile as tile
from concourse import bacc
from concourse.bass_utils import run_bass_kernel_spmd


def _install_ntff_hook():
    """The image's antenv lacks axon_hooks, so trace=True crashes in
    bass_utils. Inject a minimal antenv.axon_hooks backed by the boot
    module's ctypes NTFF profiler. No-op if anything is missing."""
    import sys, types
    try:
        import antenv
        if "antenv.axon_hooks" in sys.modules:
            return
        from trn_agent_boot.trn_boot import _ntff_profile_via_ctypes
        hook = _ntff_profile_via_ctypes("/opt/axon/libaxon_pjrt.so")
        mod = types.ModuleType("antenv.axon_hooks")
        mod.get_axon_ntff_profile_hook = lambda: hook
        mod.set_axon_ntff_profile_hook = lambda h: None
        sys.modules["antenv.axon_hooks"] = mod
        antenv.axon_hooks = mod
    except Exception:
        pass


_install_ntff_hook()

P = 128
D_MODEL = 4096
D_HIDDEN = 11008
RANK = 16
BATCH, SEQ = 2, 2048
TOK = BATCH * SEQ          # 4096 tokens
N_CORES = 8
M = TOK // N_CORES         # 512 tokens per core
KT = D_MODEL // P          # 32 contraction tiles for gate/up
HT = D_HIDDEN // P         # 86 hidden tiles
DT = D_MODEL // P          # 32 output tiles for down
CK = 2                     # k-slices per x chunk
NCH = KT // CK             # 16 x chunks
WQ = 4                     # k-slices per weight sub-tile
NSUB = KT // WQ            # 8 sub-tiles per projection tile
NWAVE = 2                  # h-tiles in the pivoted first wave

BF16 = mybir.dt.float16
F32 = mybir.dt.float32
NP_BF16 = np.float16

_NC_CACHE = {}


def _build_nc():
    nc = bacc.Bacc("TRN2")
    xt_d = nc.dram_tensor("xt", [P, KT, M], BF16, kind="ExternalInput")
    wg_d = nc.dram_tensor("wg", [HT, P, KT, P], BF16, kind="ExternalInput")
    wu_d = nc.dram_tensor("wu", [HT, P, KT, P], BF16, kind="ExternalInput")
    wd_d = nc.dram_tensor("wd", [DT, P, HT, P], BF16, kind="ExternalInput")
    ot_d = nc.dram_tensor("ot", [DT, P, M], F32, kind="ExternalOutput")

    with tile.TileContext(nc) as tc:
        with (
            tc.tile_pool(name="singles", bufs=1) as singles,
            tc.tile_pool(name="wgu", bufs=2 * NSUB) as wgu,
            tc.tile_pool(name="wdp", bufs=2) as wdp,
            tc.tile_pool(name="tmp", bufs=2) as tmpp,
            tc.tile_pool(name="ostg", bufs=2) as ostg,
            tc.tile_pool(name="occ", bufs=4) as occ,
            tc.tile_pool(name="pgu", bufs=3, space="PSUM") as pgu,
            tc.tile_pool(name="pdp", bufs=2, space="PSUM") as pdp,
        ):
            wz = singles.tile([P, M], BF16)
            xt_c = [singles.tile([P, CK, M], BF16, name=f"xc{i}")
                    for i in range(NCH)]
            hT = singles.tile([P, HT, M], BF16)
            nc.vector.memset(wz, 0)

            def xts(k):
                return xt_c[k // CK][:, k % CK, :]

            # DMA order: first wave interleaved by k-quad so the earliest
            # matmuls' operands land first.
            wsub = {}   # (proj, h, a) -> tile
            for a in range(NSUB):
                nc.sync.dma_start(out=xt_c[2 * a],
                                  in_=xt_d[:, CK * 2 * a:CK * (2 * a + 1), :])
                nc.sync.dma_start(out=xt_c[2 * a + 1],
                                  in_=xt_d[:, CK * (2 * a + 1):CK * (2 * a + 2), :])
                for h in range(NWAVE):
                    tg = wgu.tile([P, WQ, P], BF16, tag="wg")
                    nc.sync.dma_start(out=tg, in_=wg_d[h][:, WQ * a:WQ * (a + 1), :])
                    wsub[("g", h, a)] = tg
                    tu = wgu.tile([P, WQ, P], BF16, tag="wu")
                    nc.sync.dma_start(out=tu, in_=wu_d[h][:, WQ * a:WQ * (a + 1), :])
                    wsub[("u", h, a)] = tu

            # PSUM groups for the wave (2 h-tiles x gate/up)
            pg = [pgu.tile([P, M], F32, tag="pg", name=f"pgw{i}")
                  for i in range(NWAVE)]
            pu = [pgu.tile([P, M], F32, tag="pu", name=f"puw{i}")
                  for i in range(NWAVE)]

            # warmup: flip the HAM clock gate during engine/DMA spin-up;
            # 0*0 contributes exactly 0 to pg[0]'s accumulation group.
            NWARM = 12
            for i in range(NWARM):
                nc.tensor.matmul(pg[0], wz[:, 0:P], wz,
                                 start=(i == 0), stop=False)

            # ---- pivoted first wave: h=0..NWAVE-1, k-major ----
            for k in range(KT):
                a, r = k // WQ, k % WQ
                for h in range(NWAVE):
                    nc.tensor.matmul(pg[h], wsub[("g", h, a)][:, r, :], xts(k),
                                     start=(k == 0 and h != 0),
                                     stop=(k == KT - 1))
                    nc.tensor.matmul(pu[h], wsub[("u", h, a)][:, r, :], xts(k),
                                     start=(k == 0), stop=(k == KT - 1))
            for h in range(NWAVE):
                sg = tmpp.tile([P, M], F32, tag="sg")
                nc.scalar.activation(sg, pg[h],
                                     mybir.ActivationFunctionType.Silu)
                nc.vector.tensor_mul(out=hT[:, h, :], in0=sg, in1=pu[h])

            # ---- steady gate/up loop ----
            for h in range(NWAVE, HT):
                gsub = []
                usub = []
                for a in range(NSUB):
                    tg = wgu.tile([P, WQ, P], BF16, tag="wg")
                    nc.sync.dma_start(out=tg, in_=wg_d[h][:, WQ * a:WQ * (a + 1), :])
                    gsub.append(tg)
                    tu = wgu.tile([P, WQ, P], BF16, tag="wu")
                    nc.sync.dma_start(out=tu, in_=wu_d[h][:, WQ * a:WQ * (a + 1), :])
                    usub.append(tu)
                pgh = pgu.tile([P, M], F32, tag="pg")
                puh = pgu.tile([P, M], F32, tag="pu")
                for k in range(KT):
                    nc.tensor.matmul(pgh, gsub[k // WQ][:, k % WQ, :], xts(k),
                                     start=(k == 0), stop=(k == KT - 1))
                for k in range(KT):
                    nc.tensor.matmul(puh, usub[k // WQ][:, k % WQ, :], xts(k),
                                     start=(k == 0), stop=(k == KT - 1))
                sg = tmpp.tile([P, M], F32, tag="sg")
                nc.scalar.activation(sg, pgh,
                                     mybir.ActivationFunctionType.Silu)
                nc.vector.tensor_mul(out=hT[:, h, :], in0=sg, in1=puh)

            # ---- down ----
            for d in range(DT):
                wd_t = wdp.tile([P, HT, P], BF16, tag="wd")
                nc.sync.dma_start(out=wd_t, in_=wd_d[d])
                pd = pdp.tile([P, M], F32, tag="pd")
                for kh in range(HT):
                    nc.tensor.matmul(pd, wd_t[:, kh, :], hT[:, kh, :],
                                     start=(kh == 0), stop=(kh == HT - 1))
                if d < DT - 1:
                    o = ostg.tile([P, M], F32, tag="o")
                    nc.vector.tensor_copy(out=o, in_=pd)
                    nc.sync.dma_start(out=ot_d[d], in_=o)
                else:
                    # shorter drain after the final matmul: 4 column chunks
                    # alternating vector/scalar so two engines drain in
                    # parallel.
                    for c in range(4):
                        oc = occ.tile([P, P], F32, tag="oc")
                        if c % 2 == 0:
                            nc.vector.tensor_copy(out=oc,
                                                  in_=pd[:, P * c:P * (c + 1)])
                        else:
                            nc.scalar.activation(
                                oc, pd[:, P * c:P * (c + 1)],
                                mybir.ActivationFunctionType.Copy)
                        nc.sync.dma_start(out=ot_d[d, :, P * c:P * (c + 1)],
                                          in_=oc)

    nc.finalize()
    return nc


def _get_nc():
    if "nc" not in _NC_CACHE:
        _NC_CACHE["nc"] = _build_nc()
    return _NC_CACHE["nc"]


def _prepare_inputs(x, gate_w, up_w, down_w, gate_a, gate_b, up_a, up_b,
                    down_a, down_b):
    f = np.float32
    x = np.asarray(x, f).reshape(TOK, D_MODEL)
    wg = np.asarray(gate_w, f) + np.asarray(gate_b, f) @ np.asarray(gate_a, f)
    wu = np.asarray(up_w, f) + np.asarray(up_b, f) @ np.asarray(up_a, f)
    wd = np.asarray(down_w, f) + np.asarray(down_b, f) @ np.asarray(down_a, f)

    # wg_dev[h, p, k, c] = wg[h*128+c, k*128+p]
    wg_dev = np.ascontiguousarray(
        wg.reshape(HT, P, KT, P).transpose(0, 3, 2, 1)).astype(NP_BF16)
    wu_dev = np.ascontiguousarray(
        wu.reshape(HT, P, KT, P).transpose(0, 3, 2, 1)).astype(NP_BF16)
    # wd_dev[d, p, kh, c] = wd[d*128+c, kh*128+p]
    wd_dev = np.ascontiguousarray(
        wd.reshape(DT, P, HT, P).transpose(0, 3, 2, 1)).astype(NP_BF16)
    # x_dev[core, p, k, m] = x[core*512+m, k*128+p]
    x_dev = np.ascontiguousarray(
        x.reshape(N_CORES, M, KT, P).transpose(0, 3, 2, 1)).astype(NP_BF16)

    in_maps = [
        {"xt": x_dev[c], "wg": wg_dev, "wu": wu_dev, "wd": wd_dev}
        for c in range(N_CORES)
    ]
    return in_maps


def _assemble(results):
    out = np.empty((TOK, D_MODEL), np.float32)
    for c in range(N_CORES):
        oc = results[c]["ot"].reshape(D_MODEL, M)  # [d, m]
        out[c * M:(c + 1) * M, :] = oc.T
    return out.reshape(BATCH, SEQ, D_MODEL)


def run(trace=False, **inputs):
    nc = _get_nc()
    in_maps = _prepare_inputs(**inputs)
    res = run_bass_kernel_spmd(nc, in_maps, core_ids=list(range(N_CORES)),
                               trace=trace)
    return _assemble(res.results), res


def kernel(**inputs):
    out, _ = run(trace=False, **inputs)
    return out
